# revision 1
# baseline (speedup 1.0000x reference)
"""LS2T (low-rank signature transform) Trainium2 kernel.

Computes, for X:[B,T,F], kernel:[K,F,U], bias:[K,U] with B=32, T=2048,
F=512, U=128, K=10 (NUM_LEVELS=4):

    M[k] = X @ kernel[k] + bias[k]            (lift, per k)
    Y[0] = sum_t M[0]
    per level m>=2: R = M[k0]; repeat: R = M[k] * exclusive_cumsum_t(R)
    Y[m-1] = sum_t R
    out = stack(Y) : [B, NUM_LEVELS, U]

Strategy (8 NeuronCores, data-parallel over batch, 4 examples/core):
  - Host pre-transposes X to X^T [ex, fchunk, 128f, T] so the lift matmul
    contracts f on partitions with no on-device transpose.
  - Lift matmuls in float32r (full fp32 data, fast PE path) accumulate
    M[k] as [128u, T] in PSUM (4 banks), double-buffered.
  - Cumsum chains via DVE tensor_tensor_scan along the free (time) axis,
    multiplies via tensor_tensor, final multiply+time-reduce fused via
    tensor_tensor_reduce. Level-1 reduce on the Scalar engine
    (activation Copy + accumulator).
  - Y columns collected as [128u, 16] in SBUF; one PE transpose at the
    end -> [16, 128] -> DMA to DRAM.
"""

import numpy as np

import concourse.bass as bass
from concourse import bacc
import concourse.mybir as mybir
import concourse.tile as tile
from concourse.bass_utils import run_bass_kernel_spmd

# Problem constants (hardcoded per the harness contract)
B, T, F, U = 32, 2048, 512, 128
NUM_LEVELS = 4
K = NUM_LEVELS * (NUM_LEVELS + 1) // 2  # 10
NCORES = 8
EX = B // NCORES  # 4 examples per core
FCH = F // 128  # 4 f-chunks
NQ = T // 512  # 4 PSUM-bank quarters per M tile

F32 = mybir.dt.float32
F32R = mybir.dt.float32r
ALU = mybir.AluOpType
ACTF = mybir.ActivationFunctionType


def _emit(nc, tc, xT, kern, ident, outd, biasd=None, onesd=None):
    """Emit the per-core Tile program."""
    with (
        tc.tile_pool(name="const", bufs=1) as cpool,
        tc.tile_pool(name="xp", bufs=2) as xpool,
        tc.tile_pool(name="work", bufs=1) as wpool,
    ):
        # --- constants (loaded once) ---
        kt = cpool.tile([128, K * FCH * U], F32R, tag="kt", name="kt")
        nc.sync.dma_start(
            out=kt.rearrange("f (k c u) -> f k c u", k=K, c=FCH),
            in_=kern.rearrange("k c f u -> f k c u"),
        )
        idt = cpool.tile([128, 128], F32, tag="idt", name="idt")
        nc.sync.dma_start(out=idt, in_=ident[:, :])
        ones_row = cpool.tile([128, T], F32, tag="ones", name="ones_row")
        nc.gpsimd.memset(ones_row, 1.0)
        ytile = cpool.tile([128, EX * NUM_LEVELS], F32, tag="y", name="ytile")
        if biasd is not None:
            bias_sb = cpool.tile([1, K * U], F32R, tag="bias", name="bias_sb")
            nc.sync.dma_start(out=bias_sb, in_=biasd.rearrange("k u -> 1 (k u)"))
            ones_mm = cpool.tile([1, 512], F32R, tag="ones_mm", name="ones_mm")
            nc.sync.dma_start(out=ones_mm, in_=onesd[:, :])

        def kslice(k, c):
            return kt[:, (k * FCH + c) * U:(k * FCH + c + 1) * U]

        with tc.tile_pool(name="mp", bufs=2, space="PSUM") as mpool:
            for ex in range(EX):
                xt = xpool.tile([128, FCH * T], F32R, tag="xt", name=f"xt{ex}")
                for c in range(FCH):
                    nc.sync.dma_start(
                        out=xt[:, c * T:(c + 1) * T], in_=xT[ex, c]
                    )

                def lift(k):
                    m = mpool.tile([128, T], F32, tag="m", name=f"m{ex}_{k}")
                    for q in range(NQ):
                        qs = slice(q * 512, (q + 1) * 512)
                        for c in range(FCH):
                            nc.tensor.matmul(
                                m[:, qs],
                                lhsT=kslice(k, c),
                                rhs=xt[:, c * T + q * 512: c * T + (q + 1) * 512],
                                start=(c == 0),
                                stop=(c == FCH - 1 and biasd is None),
                            )
                        if biasd is not None:
                            nc.tensor.matmul(
                                m[:, qs],
                                lhsT=bias_sb[:, k * U:(k + 1) * U],
                                rhs=ones_mm,
                                start=False,
                                stop=True,
                            )
                    return m

                def scan_excl(src, nm):
                    """Exclusive cumsum of src[:, 0:T] along free axis.
                    Returns a [128, T] view whose col 0 is 0."""
                    cb = wpool.tile(
                        [128, T + 1], F32, tag="cb", bufs=3, name=f"cb_{nm}"
                    )
                    nc.gpsimd.memset(cb[:, 0:1], 0.0)
                    nc.vector.tensor_tensor_scan(
                        out=cb[:, 1:T],
                        data0=ones_row[:, 0:T - 1],
                        data1=src[:, 0:T - 1],
                        initial=0.0,
                        op0=ALU.mult,
                        op1=ALU.add,
                    )
                    return cb[:, 0:T]

                def mult(m, cview, nm):
                    # Stage M out of PSUM on the Scalar engine, multiply on
                    # GpSimd — keeps the DVE free for the scan chain.
                    ms = wpool.tile([128, T], F32, tag="ms", bufs=2, name=f"ms_{nm}")
                    nc.scalar.activation(out=ms, in_=m, func=ACTF.Copy)
                    pb = wpool.tile([128, T], F32, tag="pb", bufs=2, name=f"pb_{nm}")
                    nc.gpsimd.tensor_tensor(out=pb, in0=ms, in1=cview, op=ALU.mult)
                    return pb

                def final_reduce(m, cview, lvl, nm):
                    # Fused multiply + time-reduce in one DVE op
                    # (scalar_tensor_tensor with accumulator output; the
                    # dedicated tensor_tensor_reduce wedges the DVE at
                    # runtime in this environment).
                    sc = wpool.tile([128, T], F32, tag="sc", bufs=2, name=f"sc_{nm}")
                    ycol = ex * NUM_LEVELS + lvl
                    nc.vector.scalar_tensor_tensor(
                        out=sc,
                        in0=m,
                        scalar=1.0,
                        in1=cview,
                        op0=ALU.mult,
                        op1=ALU.mult,
                        accum_out=ytile[:, ycol:ycol + 1],
                    )

                # level 1: plain time-sum of M0, on the Scalar engine
                m0 = lift(0)
                sc0 = wpool.tile([128, T], F32, tag="sc", bufs=2, name=f"sc0_{ex}")
                nc.scalar.activation(
                    out=sc0,
                    in_=m0,
                    func=ACTF.Copy,
                    accum_out=ytile[:, ex * NUM_LEVELS:ex * NUM_LEVELS + 1],
                )
                # level 2
                m1 = lift(1)
                c = scan_excl(m1, f"{ex}_a")
                m2 = lift(2)
                final_reduce(m2, c, 1, f"{ex}_l2")
                # level 3
                m3 = lift(3)
                c = scan_excl(m3, f"{ex}_b")
                m4 = lift(4)
                p = mult(m4, c, f"{ex}_a")
                c = scan_excl(p, f"{ex}_c")
                m5 = lift(5)
                final_reduce(m5, c, 2, f"{ex}_l3")
                # level 4
                m6 = lift(6)
                c = scan_excl(m6, f"{ex}_d")
                m7 = lift(7)
                p = mult(m7, c, f"{ex}_b")
                c = scan_excl(p, f"{ex}_e")
                m8 = lift(8)
                p = mult(m8, c, f"{ex}_c")
                c = scan_excl(p, f"{ex}_f")
                m9 = lift(9)
                final_reduce(m9, c, 3, f"{ex}_l4")

        # final transpose of Y: [128u, 16] -> [16, 128u] and store
        with tc.tile_pool(name="yp", bufs=1, space="PSUM") as ypool:
            yps = ypool.tile([EX * NUM_LEVELS, 128], F32, tag="yps", name="yps")
            nc.tensor.matmul(
                yps, lhsT=ytile[:, 0:EX * NUM_LEVELS], rhs=idt,
                start=True, stop=True,
            )
            ysb = wpool.tile([EX * NUM_LEVELS, 128], F32, tag="ysb", name="ysb")
            nc.vector.tensor_copy(ysb, yps)
            nc.sync.dma_start(out=outd[:, :], in_=ysb)


def build_nc(with_bias):
    nc = bacc.Bacc(trn_type="TRN2", debug=False)
    xT = nc.dram_tensor("xT", [EX, FCH, 128, T], F32R, kind="ExternalInput")
    kern = nc.dram_tensor("kern", [K, FCH, 128, U], F32R, kind="ExternalInput")
    ident = nc.dram_tensor("ident", [128, 128], F32, kind="ExternalInput")
    biasd = onesd = None
    if with_bias:
        biasd = nc.dram_tensor("bias", [K, U], F32R, kind="ExternalInput")
        onesd = nc.dram_tensor("ones_mm", [1, 512], F32R, kind="ExternalInput")
    outd = nc.dram_tensor(
        "out", [EX * NUM_LEVELS, U], F32, kind="ExternalOutput"
    )
    with tile.TileContext(nc) as tc:
        _emit(nc, tc, xT, kern, ident, outd, biasd, onesd)
    nc.compile()
    return nc


_nc_cache = {}


def _get_nc(with_bias):
    if with_bias not in _nc_cache:
        _nc_cache[with_bias] = build_nc(with_bias)
    return _nc_cache[with_bias]


def round_fp32r(a):
    """Round fp32 to the fp32r storage format: round-to-nearest-even at
    11 mantissa bits (low 12 bits zero)."""
    b = np.ascontiguousarray(a, dtype=np.float32).view(np.uint32).copy()
    lsb = (b >> np.uint32(12)) & np.uint32(1)
    b += np.uint32(0x7FF) + lsb
    b &= np.uint32(0xFFFFF000)
    return b.view(np.float32)


def make_in_maps(X, kernel, bias, with_bias):
    kern_r = round_fp32r(kernel.reshape(K, FCH, 128, U))
    ident = np.eye(128, dtype=np.float32)
    in_maps = []
    for c in range(NCORES):
        xb = X[c * EX:(c + 1) * EX]  # [EX, T, F]
        xT = round_fp32r(np.ascontiguousarray(xb.transpose(0, 2, 1))).reshape(EX, FCH, 128, T)
        im = {"xT": xT, "kern": kern_r, "ident": ident}
        if with_bias:
            im["bias"] = round_fp32r(bias)
            im["ones_mm"] = np.ones((1, 512), np.float32)
        in_maps.append(im)
    return in_maps


def kernel(X, kernel, bias, **run_kwargs):
    X = np.asarray(X, dtype=np.float32)
    kernel = np.asarray(kernel, dtype=np.float32)
    bias = np.asarray(bias, dtype=np.float32)
    with_bias = bool(np.any(bias))
    nc = _get_nc(with_bias)
    in_maps = make_in_maps(X, kernel, bias, with_bias)
    res = run_bass_kernel_spmd(
        nc, in_maps, core_ids=list(range(NCORES)), **run_kwargs
    )
    out = np.concatenate(
        [r["out"].reshape(EX, NUM_LEVELS, U) for r in res.results], axis=0
    )
    if run_kwargs:
        return out, res
    return out



# revision 2
# speedup vs baseline: 1.2649x; 1.2649x over previous
"""LS2T (low-rank signature transform) Trainium2 kernel.

Computes, for X:[B,T,F], kernel:[K,F,U], bias:[K,U] with B=32, T=2048,
F=512, U=128, K=10 (NUM_LEVELS=4):

    M[k] = X @ kernel[k] + bias[k]            (lift, per k)
    Y[0] = sum_t M[0]
    per level m>=2: R = M[k0]; repeat: R = M[k] * exclusive_cumsum_t(R)
    Y[m-1] = sum_t R
    out = stack(Y) : [B, NUM_LEVELS, U]

Strategy (8 NeuronCores, data-parallel over batch, 4 examples/core):
  - Host pre-transposes X to X^T [ex, fchunk, 128f, T] in bf16 so the
    lift matmul contracts f on partitions with no on-device transpose.
    bf16 operands run the PE at 1 row/cycle and halve DMA traffic
    (pipeline rel err ~7e-3, well under the 2e-2 gate).
  - Lifts accumulate M[k] as [128u, T] fp32 in PSUM (4 banks,
    double-buffered); chunk-outer/quarter-inner order keeps lhsT
    resident across 4 consecutive matmuls.
  - Every M is immediately staged PSUM->SBUF as bf16 on the Scalar
    engine (~2us), so the PE never stalls on PSUM banks.
  - Cumsum chains: DVE tensor_tensor_scan (fp32 internal state, bf16
    out); chain multiplies on DVE tensor_tensor in bf16 (2x_1p mode,
    ~1.2us); final reduces split between DVE scalar_tensor_tensor
    (level 3) and GpSimd mult + Scalar accum (levels 2/4) so no engine
    exceeds the PE's ~138us/core.
  - Per example the levels run 4,1,2,3 in lift order 6,7,8,9,0,1,2,3,4,5
    so the long level-4 chain starts first and the tail stays short.
  - Y columns collect as [128u, 16]; one PE transpose -> [16, 128] ->
    DMA to DRAM.
"""

import numpy as np
import ml_dtypes

import concourse.bass as bass
from concourse import bacc
import concourse.mybir as mybir
import concourse.tile as tile
from concourse.bass_utils import run_bass_kernel_spmd

# Problem constants (hardcoded per the harness contract)
B, T, F, U = 32, 2048, 512, 128
NUM_LEVELS = 4
K = NUM_LEVELS * (NUM_LEVELS + 1) // 2  # 10
NCORES = 8
EX = B // NCORES  # 4 examples per core
FCH = F // 128  # 4 f-chunks
NQ = T // 512  # 4 PSUM-bank quarters per M tile

F32 = mybir.dt.float32
BF16 = mybir.dt.bfloat16
ALU = mybir.AluOpType
ACTF = mybir.ActivationFunctionType

LIFT_ORDER = [6, 7, 8, 9, 0, 1, 2, 3, 4, 5]


def _emit(nc, tc, xT, kt_d, ident, outd, biasd=None):
    with (
        tc.tile_pool(name="const", bufs=1) as cpool,
        tc.tile_pool(name="xp", bufs=EX) as xpool,
        tc.tile_pool(name="work", bufs=1) as wpool,
    ):
        # --- constants ---
        kt = cpool.tile([128, K * FCH * U], BF16, tag="kt", name="kt")
        nc.sync.dma_start(out=kt, in_=kt_d[:, :])
        idt = cpool.tile([128, 128], F32, tag="idt", name="idt")
        nc.sync.dma_start(out=idt, in_=ident[:, :])
        ones16 = cpool.tile([128, T], BF16, tag="ones", name="ones16")
        nc.gpsimd.memset(ones16, 1.0)
        ytile = cpool.tile([128, EX * NUM_LEVELS], F32, tag="y", name="ytile")
        if biasd is not None:
            bias_sb = cpool.tile([128, K], F32, tag="bias", name="bias_sb")
            nc.sync.dma_start(out=bias_sb, in_=biasd[:, :])

        # prefetch all X tiles up front (DMA streams while PE works)
        xts = []
        for ex in range(EX):
            xt = xpool.tile([128, FCH * T], BF16, tag="xt", name=f"xt{ex}")
            for c in range(FCH):
                nc.sync.dma_start(out=xt[:, c * T:(c + 1) * T], in_=xT[ex, c])
            xts.append(xt)

        def kslice(k, c):
            return kt[:, (k * FCH + c) * U:(k * FCH + c + 1) * U]

        with tc.tile_pool(name="mp", bufs=2, space="PSUM") as mpool:
            for ex in range(EX):
                xt = xts[ex]

                def lift(k):
                    m = mpool.tile([128, T], F32, tag="m", name=f"m{ex}_{k}")
                    for c in range(FCH):
                        for q in range(NQ):
                            nc.tensor.matmul(
                                m[:, q * 512:(q + 1) * 512],
                                lhsT=kslice(k, c),
                                rhs=xt[:, c * T + q * 512: c * T + (q + 1) * 512],
                                start=(c == 0),
                                stop=(c == FCH - 1),
                            )
                    return m

                def stage(k, m, accum_col=None):
                    """PSUM fp32 -> SBUF bf16 on Scalar; optional Y accum."""
                    ms = wpool.tile(
                        [128, T], BF16, tag="ms", bufs=6, name=f"ms{ex}_{k}"
                    )
                    kw = {}
                    if accum_col is not None:
                        kw["accum_out"] = ytile[:, accum_col:accum_col + 1]
                    if biasd is not None:
                        nc.scalar.activation(
                            out=ms, in_=m, func=ACTF.Identity,
                            bias=bias_sb[:, k:k + 1], **kw,
                        )
                    else:
                        nc.scalar.activation(out=ms, in_=m, func=ACTF.Copy, **kw)
                    return ms

                def scan_excl(src, nm):
                    """Exclusive cumsum along t (DVE, fp32 state, bf16 out)."""
                    cb = wpool.tile(
                        [128, T + 1], BF16, tag="cb", bufs=3, name=f"cb_{nm}"
                    )
                    nc.gpsimd.memset(cb[:, 0:1], 0.0)
                    nc.vector.tensor_tensor_scan(
                        out=cb[:, 1:T],
                        data0=ones16[:, 0:T - 1],
                        data1=src[:, 0:T - 1],
                        initial=0.0,
                        op0=ALU.mult,
                        op1=ALU.add,
                    )
                    return cb[:, 0:T]

                def vmult(a, c, nm):
                    """bf16 chain multiply on DVE (2x_1p)."""
                    p = wpool.tile([128, T], BF16, tag="pb", bufs=3, name=f"pb_{nm}")
                    nc.vector.tensor_tensor(out=p, in0=a, in1=c, op=ALU.mult)
                    return p

                def gmult(a, c, nm):
                    """bf16 reduce multiply on GpSimd."""
                    p = wpool.tile([128, T], BF16, tag="rb", bufs=2, name=f"rb_{nm}")
                    nc.gpsimd.tensor_tensor(out=p, in0=a, in1=c, op=ALU.mult)
                    return p

                def stt_reduce(a, c, lvl, nm):
                    """Fused multiply + t-reduce on DVE."""
                    sc = wpool.tile([128, T], BF16, tag="sc", bufs=2, name=f"sc_{nm}")
                    ycol = ex * NUM_LEVELS + lvl
                    nc.vector.scalar_tensor_tensor(
                        out=sc, in0=a, scalar=1.0, in1=c,
                        op0=ALU.mult, op1=ALU.mult,
                        accum_out=ytile[:, ycol:ycol + 1],
                    )

                def sc_reduce(r, lvl, nm):
                    """t-reduce of an SBUF bf16 tile on Scalar."""
                    sc = wpool.tile([128, T], BF16, tag="sr", bufs=2, name=f"sr_{nm}")
                    ycol = ex * NUM_LEVELS + lvl
                    nc.scalar.activation(
                        out=sc, in_=r, func=ACTF.Copy,
                        accum_out=ytile[:, ycol:ycol + 1],
                    )

                # level 4 first (longest chain), then 1, 2, 3
                m6 = lift(6)
                m6s = stage(6, m6)
                m7 = lift(7)
                m7s = stage(7, m7)
                c = scan_excl(m6s, f"{ex}c6")
                m8 = lift(8)
                m8s = stage(8, m8)
                p = vmult(m7s, c, f"{ex}p7")
                m9 = lift(9)
                m9s = stage(9, m9)
                c = scan_excl(p, f"{ex}c7")
                p = vmult(m8s, c, f"{ex}p8")
                m0 = lift(0)
                stage(0, m0, accum_col=ex * NUM_LEVELS + 0)  # level 1
                c = scan_excl(p, f"{ex}c8")
                r9 = gmult(m9s, c, f"{ex}r9")
                m1 = lift(1)
                m1s = stage(1, m1)
                m2 = lift(2)
                m2s = stage(2, m2)
                c1 = scan_excl(m1s, f"{ex}c1")
                r2 = gmult(m2s, c1, f"{ex}r2")
                m3 = lift(3)
                m3s = stage(3, m3)
                m4 = lift(4)
                m4s = stage(4, m4)
                c3 = scan_excl(m3s, f"{ex}c3")
                p4 = vmult(m4s, c3, f"{ex}p4")
                m5 = lift(5)
                m5s = stage(5, m5)
                c4 = scan_excl(p4, f"{ex}c4")
                stt_reduce(m5s, c4, 2, f"{ex}y3")
                # Scalar-queue tails: reduces after all stages of this ex
                sc_reduce(r9, 3, f"{ex}y4")
                sc_reduce(r2, 1, f"{ex}y2")

        # final transpose of Y: [128u, 16] -> [16, 128u] and store
        with tc.tile_pool(name="yp", bufs=1, space="PSUM") as ypool:
            yps = ypool.tile([EX * NUM_LEVELS, 128], F32, tag="yps", name="yps")
            nc.tensor.matmul(
                yps, lhsT=ytile[:, 0:EX * NUM_LEVELS], rhs=idt,
                start=True, stop=True,
            )
            ysb = wpool.tile([EX * NUM_LEVELS, 128], F32, tag="ysb", name="ysb")
            nc.vector.tensor_copy(ysb, yps)
            nc.sync.dma_start(out=outd[:, :], in_=ysb)


def build_nc(with_bias):
    nc = bacc.Bacc(trn_type="TRN2", debug=False)
    xT = nc.dram_tensor("xT", [EX, FCH, 128, T], BF16, kind="ExternalInput")
    kt_d = nc.dram_tensor("kt", [128, K * FCH * U], BF16, kind="ExternalInput")
    ident = nc.dram_tensor("ident", [128, 128], F32, kind="ExternalInput")
    biasd = None
    if with_bias:
        biasd = nc.dram_tensor("biasT", [128, K], F32, kind="ExternalInput")
    outd = nc.dram_tensor(
        "out", [EX * NUM_LEVELS, U], F32, kind="ExternalOutput"
    )
    with tile.TileContext(nc) as tc:
        _emit(nc, tc, xT, kt_d, ident, outd, biasd)
    nc.compile()
    return nc


_nc_cache = {}


def _get_nc(with_bias):
    if with_bias not in _nc_cache:
        _nc_cache[with_bias] = build_nc(with_bias)
    return _nc_cache[with_bias]


def make_in_maps(X, kernel, bias, with_bias):
    bf = ml_dtypes.bfloat16
    kt = np.ascontiguousarray(
        kernel.reshape(K, FCH, 128, U).transpose(2, 0, 1, 3)
    ).reshape(128, K * FCH * U).astype(bf)
    ident = np.eye(128, dtype=np.float32)
    Xb = X.astype(bf)  # [B, T, F]
    in_maps = []
    for c in range(NCORES):
        xb = Xb[c * EX:(c + 1) * EX]  # [EX, T, F] bf16
        xT = np.ascontiguousarray(xb.transpose(0, 2, 1)).reshape(EX, FCH, 128, T)
        im = {"xT": xT, "kt": kt, "ident": ident}
        if with_bias:
            im["biasT"] = np.ascontiguousarray(bias.T).astype(np.float32)
        in_maps.append(im)
    return in_maps


def kernel(X, kernel, bias, **run_kwargs):
    X = np.asarray(X, dtype=np.float32)
    kernel = np.asarray(kernel, dtype=np.float32)
    bias = np.asarray(bias, dtype=np.float32)
    with_bias = bool(np.any(bias))
    nc = _get_nc(with_bias)
    in_maps = make_in_maps(X, kernel, bias, with_bias)
    res = run_bass_kernel_spmd(
        nc, in_maps, core_ids=list(range(NCORES)), **run_kwargs
    )
    out = np.concatenate(
        [r["out"].reshape(EX, NUM_LEVELS, U) for r in res.results], axis=0
    )
    if run_kwargs:
        return out, res
    return out


# revision 5
# speedup vs baseline: 1.2680x; 1.0025x over previous
"""LS2T (low-rank signature transform) Trainium2 kernel.

Computes, for X:[B,T,F], kernel:[K,F,U], bias:[K,U] with B=32, T=2048,
F=512, U=128, K=10 (NUM_LEVELS=4):

    M[k] = X @ kernel[k] + bias[k]            (lift, per k)
    Y[0] = sum_t M[0]
    per level m>=2: R = M[k0]; repeat: R = M[k] * exclusive_cumsum_t(R)
    Y[m-1] = sum_t R
    out = stack(Y) : [B, NUM_LEVELS, U]

Strategy (8 NeuronCores, data-parallel over batch, 4 examples/core):
  - Host pre-transposes X to X^T [ex, fchunk, 128f, T] in bf16 so the
    lift matmul contracts f on partitions with no on-device transpose.
    bf16 operands run the PE at 1 row/cycle and halve DMA traffic
    (pipeline rel err ~7e-3, well under the 2e-2 gate).
  - Lifts accumulate M[k] as [128u, T] fp32 in PSUM (4 banks,
    double-buffered); chunk-outer/quarter-inner order keeps lhsT
    resident across 4 consecutive matmuls.
  - Every M is immediately staged PSUM->SBUF as bf16 on the Scalar
    engine (~2us), so the PE never stalls on PSUM banks.
  - Cumsum chains: DVE tensor_tensor_scan (fp32 internal state, bf16
    out); chain multiplies on DVE tensor_tensor in bf16 (2x_1p mode,
    ~1.2us); final reduces split between DVE scalar_tensor_tensor
    (level 3) and GpSimd mult + Scalar accum (levels 2/4) so no engine
    exceeds the PE's ~138us/core.
  - Per example the levels run 4,1,2,3 in lift order 6,7,8,9,0,1,2,3,4,5
    so the long level-4 chain starts first and the tail stays short.
  - Y columns collect as [128u, 16]; one PE transpose -> [16, 128] ->
    DMA to DRAM.
"""

import numpy as np
import ml_dtypes

import concourse.bass as bass
from concourse import bacc
import concourse.mybir as mybir
import concourse.tile as tile
from concourse.bass_utils import run_bass_kernel_spmd

# Problem constants (hardcoded per the harness contract)
B, T, F, U = 32, 2048, 512, 128
NUM_LEVELS = 4
K = NUM_LEVELS * (NUM_LEVELS + 1) // 2  # 10
NCORES = 8
EX = B // NCORES  # 4 examples per core
FCH = F // 128  # 4 f-chunks
NQ = T // 512  # 4 PSUM-bank quarters per M tile

F32 = mybir.dt.float32
BF16 = mybir.dt.bfloat16
ALU = mybir.AluOpType
ACTF = mybir.ActivationFunctionType

LIFT_ORDER = [6, 7, 8, 9, 0, 1, 2, 3, 4, 5]


def _emit(nc, tc, xT, kt_d, ident, outd, biasd=None):
    with (
        tc.tile_pool(name="const", bufs=1) as cpool,
        tc.tile_pool(name="xp", bufs=EX) as xpool,
        tc.tile_pool(name="work", bufs=1) as wpool,
    ):
        # --- constants ---
        # kt DMA split per k, first-needed k first, so lift 0 starts early
        kt = cpool.tile([128, K * FCH * U], BF16, tag="kt", name="kt")

        def kdma(k):
            s = slice(k * FCH * U, (k + 1) * FCH * U)
            nc.sync.dma_start(out=kt[:, s], in_=kt_d[:, s])

        kdma(LIFT_ORDER[0])
        xts = []
        xt0 = xpool.tile([128, FCH * T], BF16, tag="xt", name="xt0")
        for c in range(FCH):
            nc.sync.dma_start(out=xt0[:, c * T:(c + 1) * T], in_=xT[0, c])
        xts.append(xt0)
        for k in LIFT_ORDER[1:]:
            kdma(k)
        idt = cpool.tile([128, 128], F32, tag="idt", name="idt")
        nc.sync.dma_start(out=idt, in_=ident[:, :])
        ones16 = cpool.tile([128, T], BF16, tag="ones", name="ones16")
        nc.gpsimd.memset(ones16, 1.0)
        ytile = cpool.tile([128, EX * NUM_LEVELS], F32, tag="y", name="ytile")
        if biasd is not None:
            bias_sb = cpool.tile([128, K], F32, tag="bias", name="bias_sb")
            nc.sync.dma_start(out=bias_sb, in_=biasd[:, :])

        # prefetch the remaining X tiles (DMA streams while PE works)
        for ex in range(1, EX):
            xt = xpool.tile([128, FCH * T], BF16, tag="xt", name=f"xt{ex}")
            for c in range(FCH):
                nc.sync.dma_start(out=xt[:, c * T:(c + 1) * T], in_=xT[ex, c])
            xts.append(xt)

        def kslice(k, c):
            return kt[:, (k * FCH + c) * U:(k * FCH + c + 1) * U]

        with tc.tile_pool(name="mp", bufs=2, space="PSUM") as mpool:
            for ex in range(EX):
                xt = xts[ex]

                def lift(k):
                    m = mpool.tile([128, T], F32, tag="m", name=f"m{ex}_{k}")
                    for c in range(FCH):
                        for q in range(NQ):
                            nc.tensor.matmul(
                                m[:, q * 512:(q + 1) * 512],
                                lhsT=kslice(k, c),
                                rhs=xt[:, c * T + q * 512: c * T + (q + 1) * 512],
                                start=(c == 0),
                                stop=(c == FCH - 1),
                            )
                    return m

                def stage(k, m, accum_col=None):
                    """PSUM fp32 -> SBUF bf16 on Scalar; optional Y accum."""
                    ms = wpool.tile(
                        [128, T], BF16, tag="ms", bufs=6, name=f"ms{ex}_{k}"
                    )
                    kw = {}
                    if accum_col is not None:
                        kw["accum_out"] = ytile[:, accum_col:accum_col + 1]
                    if biasd is not None:
                        nc.scalar.activation(
                            out=ms, in_=m, func=ACTF.Identity,
                            bias=bias_sb[:, k:k + 1], **kw,
                        )
                    else:
                        nc.scalar.activation(out=ms, in_=m, func=ACTF.Copy, **kw)
                    return ms

                def scan_excl(src, nm):
                    """Exclusive cumsum along t (DVE, fp32 state, bf16 out)."""
                    cb = wpool.tile(
                        [128, T + 1], BF16, tag="cb", bufs=3, name=f"cb_{nm}"
                    )
                    nc.gpsimd.memset(cb[:, 0:1], 0.0)
                    nc.vector.tensor_tensor_scan(
                        out=cb[:, 1:T],
                        data0=ones16[:, 0:T - 1],
                        data1=src[:, 0:T - 1],
                        initial=0.0,
                        op0=ALU.mult,
                        op1=ALU.add,
                    )
                    return cb[:, 0:T]

                def gmult(a, c, nm):
                    """bf16 chain multiply on GpSimd (keeps DVE for scans)."""
                    p = wpool.tile([128, T], BF16, tag="pb", bufs=3, name=f"pb_{nm}")
                    nc.gpsimd.tensor_tensor(out=p, in0=a, in1=c, op=ALU.mult)
                    return p

                def stt_reduce(a, c, lvl, nm):
                    """Fused multiply + t-reduce on DVE."""
                    sc = wpool.tile([128, T], BF16, tag="sc", bufs=2, name=f"sc_{nm}")
                    ycol = ex * NUM_LEVELS + lvl
                    nc.vector.scalar_tensor_tensor(
                        out=sc, in0=a, scalar=1.0, in1=c,
                        op0=ALU.mult, op1=ALU.mult,
                        accum_out=ytile[:, ycol:ycol + 1],
                    )

                # level 4 first (longest chain), then 1, 2, 3
                m6 = lift(6)
                m6s = stage(6, m6)
                m7 = lift(7)
                m7s = stage(7, m7)
                c = scan_excl(m6s, f"{ex}c6")
                m8 = lift(8)
                m8s = stage(8, m8)
                p = gmult(m7s, c, f"{ex}p7")
                m9 = lift(9)
                m9s = stage(9, m9)
                c = scan_excl(p, f"{ex}c7")
                p = gmult(m8s, c, f"{ex}p8")
                m0 = lift(0)
                stage(0, m0, accum_col=ex * NUM_LEVELS + 0)  # level 1
                c = scan_excl(p, f"{ex}c8")
                stt_reduce(m9s, c, 3, f"{ex}y4")
                m1 = lift(1)
                m1s = stage(1, m1)
                m2 = lift(2)
                m2s = stage(2, m2)
                c1 = scan_excl(m1s, f"{ex}c1")
                stt_reduce(m2s, c1, 1, f"{ex}y2")
                m3 = lift(3)
                m3s = stage(3, m3)
                m4 = lift(4)
                m4s = stage(4, m4)
                c3 = scan_excl(m3s, f"{ex}c3")
                p4 = gmult(m4s, c3, f"{ex}p4")
                m5 = lift(5)
                m5s = stage(5, m5)
                c4 = scan_excl(p4, f"{ex}c4")
                stt_reduce(m5s, c4, 2, f"{ex}y3")

        # final transpose of Y: [128u, 16] -> [16, 128u] and store
        with tc.tile_pool(name="yp", bufs=1, space="PSUM") as ypool:
            yps = ypool.tile([EX * NUM_LEVELS, 128], F32, tag="yps", name="yps")
            nc.tensor.matmul(
                yps, lhsT=ytile[:, 0:EX * NUM_LEVELS], rhs=idt,
                start=True, stop=True,
            )
            ysb = wpool.tile([EX * NUM_LEVELS, 128], F32, tag="ysb", name="ysb")
            nc.vector.tensor_copy(ysb, yps)
            nc.sync.dma_start(out=outd[:, :], in_=ysb)


def build_nc(with_bias):
    nc = bacc.Bacc(trn_type="TRN2", debug=False)
    xT = nc.dram_tensor("xT", [EX, FCH, 128, T], BF16, kind="ExternalInput")
    kt_d = nc.dram_tensor("kt", [128, K * FCH * U], BF16, kind="ExternalInput")
    ident = nc.dram_tensor("ident", [128, 128], F32, kind="ExternalInput")
    biasd = None
    if with_bias:
        biasd = nc.dram_tensor("biasT", [128, K], F32, kind="ExternalInput")
    outd = nc.dram_tensor(
        "out", [EX * NUM_LEVELS, U], F32, kind="ExternalOutput"
    )
    with tile.TileContext(nc) as tc:
        _emit(nc, tc, xT, kt_d, ident, outd, biasd)
    nc.compile()
    return nc


_nc_cache = {}


def _get_nc(with_bias):
    if with_bias not in _nc_cache:
        _nc_cache[with_bias] = build_nc(with_bias)
    return _nc_cache[with_bias]


def make_in_maps(X, kernel, bias, with_bias):
    bf = ml_dtypes.bfloat16
    kt = np.ascontiguousarray(
        kernel.reshape(K, FCH, 128, U).transpose(2, 0, 1, 3)
    ).reshape(128, K * FCH * U).astype(bf)
    ident = np.eye(128, dtype=np.float32)
    Xb = X.astype(bf)  # [B, T, F]
    in_maps = []
    for c in range(NCORES):
        xb = Xb[c * EX:(c + 1) * EX]  # [EX, T, F] bf16
        xT = np.ascontiguousarray(xb.transpose(0, 2, 1)).reshape(EX, FCH, 128, T)
        im = {"xT": xT, "kt": kt, "ident": ident}
        if with_bias:
            im["biasT"] = np.ascontiguousarray(bias.T).astype(np.float32)
        in_maps.append(im)
    return in_maps


def kernel(X, kernel, bias, **run_kwargs):
    X = np.asarray(X, dtype=np.float32)
    kernel = np.asarray(kernel, dtype=np.float32)
    bias = np.asarray(bias, dtype=np.float32)
    with_bias = bool(np.any(bias))
    nc = _get_nc(with_bias)
    in_maps = make_in_maps(X, kernel, bias, with_bias)
    res = run_bass_kernel_spmd(
        nc, in_maps, core_ids=list(range(NCORES)), **run_kwargs
    )
    out = np.concatenate(
        [r["out"].reshape(EX, NUM_LEVELS, U) for r in res.results], axis=0
    )
    if run_kwargs:
        return out, res
    return out


# revision 9
# speedup vs baseline: 1.2965x; 1.0225x over previous
"""LS2T (low-rank signature transform) Trainium2 kernel.

Computes, for X:[B,T,F], kernel:[K,F,U], bias:[K,U] with B=32, T=2048,
F=512, U=128, K=10 (NUM_LEVELS=4):

    M[k] = X @ kernel[k] + bias[k]            (lift, per k)
    Y[0] = sum_t M[0]
    per level m>=2: R = M[k0]; repeat: R = M[k] * exclusive_cumsum_t(R)
    Y[m-1] = sum_t R
    out = stack(Y) : [B, NUM_LEVELS, U]

Strategy (8 NeuronCores, data-parallel over batch, 4 examples/core):
  - Host pre-transposes X to X^T [ex, fchunk, 128f, T] in bf16 so the
    lift matmul contracts f on partitions with no on-device transpose.
    bf16 operands run the PE at 1 row/cycle and halve DMA traffic
    (pipeline rel err ~7e-3, well under the 2e-2 gate).
  - Lifts accumulate M[k] as [128u, T] fp32 in PSUM (4 banks,
    double-buffered); chunk-outer/quarter-inner order keeps lhsT
    resident across 4 consecutive matmuls.
  - Every M is immediately staged PSUM->SBUF as bf16 on the Scalar
    engine (~2us), so the PE never stalls on PSUM banks.
  - Cumsum chains: DVE tensor_tensor_scan (fp32 internal state, bf16
    out); chain multiplies on DVE tensor_tensor in bf16 (2x_1p mode,
    ~1.2us); final reduces split between DVE scalar_tensor_tensor
    (level 3) and GpSimd mult + Scalar accum (levels 2/4) so no engine
    exceeds the PE's ~138us/core.
  - Per example the levels run 4,1,2,3 in lift order 6,7,8,9,0,1,2,3,4,5
    so the long level-4 chain starts first and the tail stays short.
  - Y columns collect as [128u, 16]; one PE transpose -> [16, 128] ->
    DMA to DRAM.
"""

import numpy as np
import ml_dtypes

import concourse.bass as bass
from concourse import bacc
import concourse.mybir as mybir
import concourse.tile as tile
from concourse.bass_utils import run_bass_kernel_spmd

# Problem constants (hardcoded per the harness contract)
B, T, F, U = 32, 2048, 512, 128
NUM_LEVELS = 4
K = NUM_LEVELS * (NUM_LEVELS + 1) // 2  # 10
NCORES = 8
EX = B // NCORES  # 4 examples per core
FCH = F // 128  # 4 f-chunks
NQ = T // 512  # 4 PSUM-bank quarters per M tile

F32 = mybir.dt.float32
BF16 = mybir.dt.bfloat16
ALU = mybir.AluOpType
ACTF = mybir.ActivationFunctionType

LIFT_ORDER = [6, 7, 8, 9, 0, 1, 2, 3, 4, 5]


def _emit(nc, tc, xT, kt_d, ident, outd, biasd=None):
    with (
        tc.tile_pool(name="const", bufs=1) as cpool,
        tc.tile_pool(name="xp", bufs=EX) as xpool,
        tc.tile_pool(name="work", bufs=1) as wpool,
    ):
        # --- constants ---
        # kt DMA split per k, first-needed k first, so lift 0 starts early
        kt = cpool.tile([128, K * FCH * U], BF16, tag="kt", name="kt")

        def kdma(k):
            s = slice(k * FCH * U, (k + 1) * FCH * U)
            nc.sync.dma_start(out=kt[:, s], in_=kt_d[:, s])

        kdma(LIFT_ORDER[0])
        xts = []
        xt0 = xpool.tile([128, FCH * T], BF16, tag="xt", name="xt0")
        for c in range(FCH):
            nc.sync.dma_start(out=xt0[:, c * T:(c + 1) * T], in_=xT[0, c])
        xts.append(xt0)
        for k in LIFT_ORDER[1:]:
            kdma(k)
        idt = cpool.tile([128, 128], F32, tag="idt", name="idt")
        nc.sync.dma_start(out=idt, in_=ident[:, :])
        ones16 = cpool.tile([128, T], BF16, tag="ones", name="ones16")
        nc.gpsimd.memset(ones16, 1.0)
        ytile = cpool.tile([128, EX * NUM_LEVELS], F32, tag="y", name="ytile")
        if biasd is not None:
            bias_sb = cpool.tile([128, K], F32, tag="bias", name="bias_sb")
            nc.sync.dma_start(out=bias_sb, in_=biasd[:, :])

        # prefetch the remaining X tiles (DMA streams while PE works)
        for ex in range(1, EX):
            xt = xpool.tile([128, FCH * T], BF16, tag="xt", name=f"xt{ex}")
            for c in range(FCH):
                nc.sync.dma_start(out=xt[:, c * T:(c + 1) * T], in_=xT[ex, c])
            xts.append(xt)

        def kslice(k, c):
            return kt[:, (k * FCH + c) * U:(k * FCH + c + 1) * U]

        with tc.tile_pool(name="mp", bufs=2, space="PSUM") as mpool:
            for ex in range(EX):
                xt = xts[ex]

                def lift(k):
                    m = mpool.tile([128, T], F32, tag="m", name=f"m{ex}_{k}")
                    for c in range(FCH):
                        for q in range(NQ):
                            nc.tensor.matmul(
                                m[:, q * 512:(q + 1) * 512],
                                lhsT=kslice(k, c),
                                rhs=xt[:, c * T + q * 512: c * T + (q + 1) * 512],
                                start=(c == 0),
                                stop=(c == FCH - 1),
                            )
                    return m

                def stage(k, m, accum_col=None):
                    """PSUM fp32 -> SBUF bf16 on Scalar; optional Y accum."""
                    ms = wpool.tile(
                        [128, T], BF16, tag="ms", bufs=10, name=f"ms{ex}_{k}"
                    )
                    kw = {}
                    if accum_col is not None:
                        kw["accum_out"] = ytile[:, accum_col:accum_col + 1]
                    if biasd is not None:
                        nc.scalar.activation(
                            out=ms, in_=m, func=ACTF.Identity,
                            bias=bias_sb[:, k:k + 1], **kw,
                        )
                    else:
                        nc.scalar.activation(out=ms, in_=m, func=ACTF.Copy, **kw)
                    return ms

                def scan_excl(src, nm):
                    """Exclusive cumsum along t (DVE, fp32 state, bf16 out)."""
                    cb = wpool.tile(
                        [128, T + 1], BF16, tag="cb", bufs=4, name=f"cb_{nm}"
                    )
                    nc.gpsimd.memset(cb[:, 0:1], 0.0)
                    nc.vector.tensor_tensor_scan(
                        out=cb[:, 1:T],
                        data0=ones16[:, 0:T - 1],
                        data1=src[:, 0:T - 1],
                        initial=0.0,
                        op0=ALU.mult,
                        op1=ALU.add,
                    )
                    return cb[:, 0:T]

                def vmult(a, c, nm):
                    """bf16 chain multiply on DVE (2x_1p, ~1.2us)."""
                    p = wpool.tile([128, T], BF16, tag="pb", bufs=3, name=f"pb_{nm}")
                    nc.vector.tensor_tensor(out=p, in0=a, in1=c, op=ALU.mult)
                    return p

                def gmult(a, c, nm):
                    """bf16 last-factor multiply on GpSimd (off DVE's path)."""
                    p = wpool.tile([128, T], BF16, tag="rb", bufs=3, name=f"rb_{nm}")
                    nc.gpsimd.tensor_tensor(out=p, in0=a, in1=c, op=ALU.mult)
                    return p

                def sc_accum(r, lvl, nm):
                    """t-reduce of an SBUF bf16 tile on Scalar. Wait-pinned
                    late in sim time so the scheduler queues it after this
                    example's stages (avoids head-of-line blocking)."""
                    sc = wpool.tile([128, T], BF16, tag="sr", bufs=2, name=f"sr_{nm}")
                    ycol = ex * NUM_LEVELS + lvl
                    with tc.tile_wait_until(0.045 + 0.035 * ex):
                        nc.scalar.activation(
                            out=sc, in_=r, func=ACTF.Copy,
                            accum_out=ytile[:, ycol:ycol + 1],
                        )

                # level 4 first (longest chain), then 1, 2, 3
                m6 = lift(6)
                m6s = stage(6, m6)
                m7 = lift(7)
                m7s = stage(7, m7)
                c = scan_excl(m6s, f"{ex}c6")
                m8 = lift(8)
                m8s = stage(8, m8)
                p = vmult(m7s, c, f"{ex}p7")
                m9 = lift(9)
                m9s = stage(9, m9)
                c = scan_excl(p, f"{ex}c7")
                p = vmult(m8s, c, f"{ex}p8")
                m0 = lift(0)
                stage(0, m0, accum_col=ex * NUM_LEVELS + 0)  # level 1
                c = scan_excl(p, f"{ex}c8")
                r9 = gmult(m9s, c, f"{ex}r9")
                sc_accum(r9, 3, f"{ex}y4")
                m1 = lift(1)
                m1s = stage(1, m1)
                m2 = lift(2)
                m2s = stage(2, m2)
                c1 = scan_excl(m1s, f"{ex}c1")
                r2 = gmult(m2s, c1, f"{ex}r2")
                sc_accum(r2, 1, f"{ex}y2")
                m3 = lift(3)
                m3s = stage(3, m3)
                m4 = lift(4)
                m4s = stage(4, m4)
                c3 = scan_excl(m3s, f"{ex}c3")
                p4 = vmult(m4s, c3, f"{ex}p4")
                m5 = lift(5)
                m5s = stage(5, m5)
                c4 = scan_excl(p4, f"{ex}c4")
                r5 = gmult(m5s, c4, f"{ex}r5")
                sc_accum(r5, 2, f"{ex}y3")

        # final transpose of Y: [128u, 16] -> [16, 128u] and store
        with tc.tile_pool(name="yp", bufs=1, space="PSUM") as ypool:
            yps = ypool.tile([EX * NUM_LEVELS, 128], F32, tag="yps", name="yps")
            nc.tensor.matmul(
                yps, lhsT=ytile[:, 0:EX * NUM_LEVELS], rhs=idt,
                start=True, stop=True,
            )
            ysb = wpool.tile([EX * NUM_LEVELS, 128], F32, tag="ysb", name="ysb")
            nc.vector.tensor_copy(ysb, yps)
            nc.sync.dma_start(out=outd[:, :], in_=ysb)


def build_nc(with_bias):
    nc = bacc.Bacc(trn_type="TRN2", debug=False)
    xT = nc.dram_tensor("xT", [EX, FCH, 128, T], BF16, kind="ExternalInput")
    kt_d = nc.dram_tensor("kt", [128, K * FCH * U], BF16, kind="ExternalInput")
    ident = nc.dram_tensor("ident", [128, 128], F32, kind="ExternalInput")
    biasd = None
    if with_bias:
        biasd = nc.dram_tensor("biasT", [128, K], F32, kind="ExternalInput")
    outd = nc.dram_tensor(
        "out", [EX * NUM_LEVELS, U], F32, kind="ExternalOutput"
    )
    with tile.TileContext(nc) as tc:
        _emit(nc, tc, xT, kt_d, ident, outd, biasd)
    nc.compile()
    return nc


_nc_cache = {}


def _get_nc(with_bias):
    if with_bias not in _nc_cache:
        _nc_cache[with_bias] = build_nc(with_bias)
    return _nc_cache[with_bias]


def make_in_maps(X, kernel, bias, with_bias):
    bf = ml_dtypes.bfloat16
    kt = np.ascontiguousarray(
        kernel.reshape(K, FCH, 128, U).transpose(2, 0, 1, 3)
    ).reshape(128, K * FCH * U).astype(bf)
    ident = np.eye(128, dtype=np.float32)
    Xb = X.astype(bf)  # [B, T, F]
    in_maps = []
    for c in range(NCORES):
        xb = Xb[c * EX:(c + 1) * EX]  # [EX, T, F] bf16
        xT = np.ascontiguousarray(xb.transpose(0, 2, 1)).reshape(EX, FCH, 128, T)
        im = {"xT": xT, "kt": kt, "ident": ident}
        if with_bias:
            im["biasT"] = np.ascontiguousarray(bias.T).astype(np.float32)
        in_maps.append(im)
    return in_maps


def kernel(X, kernel, bias, **run_kwargs):
    X = np.asarray(X, dtype=np.float32)
    kernel = np.asarray(kernel, dtype=np.float32)
    bias = np.asarray(bias, dtype=np.float32)
    with_bias = bool(np.any(bias))
    nc = _get_nc(with_bias)
    in_maps = make_in_maps(X, kernel, bias, with_bias)
    res = run_bass_kernel_spmd(
        nc, in_maps, core_ids=list(range(NCORES)), **run_kwargs
    )
    out = np.concatenate(
        [r["out"].reshape(EX, NUM_LEVELS, U) for r in res.results], axis=0
    )
    if run_kwargs:
        return out, res
    return out
